# revision 1
# baseline (speedup 1.0000x reference)
"""Trainium2 Bass kernel for nn_CSATransformer_25778393710760.

Math: with the given parameters (all biases zero, ln identity) the module
reduces exactly to
    out = LN(relu(x @ pfn_w1) @ pfn_w2 + x)
(LayerNorm cancels the positive per-row colsum scale; see git history).

Device kernel (per core, one batch example, L=4096 rows, D=128), bf16
matmul path with the LayerNorm centering folded into the weights:
  - host passes xb = bf16(x) and xcb = bf16(x - rowmean(x)), plus
    W2C = pfn_w2 @ (I - J/128) so the PFF output is row-centered.
  - po = xcb + relu(xb@W1)@W2C accumulated in PSUM fp32 is then exactly
    y - rowmean(y), so LN reduces to po * rsqrt(mean(po^2) + eps).
  - row mapping: row = 32p + 4g + r (p = partition, slab g, r = 0..3).
    PE per slab: 4 input transposes, one 512-wide matmul-1, one 512-wide
    residual pass-through (ident stationary), 4 chunk matmul-2s with the
    relu'd intermediate stationary so output lands row-major.
  - LN: drain po once to SBUF bf16, square on GpSimd, grouped reduce +
    broadcast-normalize on DVE at 16-bit rates; og stays bf16 and the
    SWDGE store casts to fp32 on the way to HBM.
"""

import os
import numpy as np

B, L, DX = 8, 4096, 128
_SLABS = 8          # 512-row slabs per core
_R = 4              # chunks (rows per partition) per slab

_prog_cache = {}


def _build_program():
    import concourse.tile as tile
    from concourse import bacc, mybir
    from concourse.bass import ts

    f32 = mybir.dt.float32
    bf16 = mybir.dt.bfloat16
    AF = mybir.ActivationFunctionType
    OP = mybir.AluOpType

    nc = bacc.Bacc(None, target_bir_lowering=False)
    xb = nc.dram_tensor("xb", [L, DX], bf16, kind="ExternalInput")
    # xcb host-rearranged to [slab, partition, chunk, d] with the
    # row = 32p + 4g + r mapping: per-partition slab data is 1KB-contiguous.
    xcb = nc.dram_tensor("xcb", [_SLABS, 128, _R, DX], bf16,
                         kind="ExternalInput")
    wpack = nc.dram_tensor("wpack", [DX, 3 * DX], bf16, kind="ExternalInput")
    y = nc.dram_tensor("y", [L, DX], f32, kind="ExternalOutput")

    xb_r = xb.rearrange("(p k) d -> p k d", p=128)
    y_r = y.rearrange("(p k) d -> p k d", p=128)

    with tile.TileContext(nc) as tc:
        with (
            tc.tile_pool(name="consts", bufs=1) as consts,
            tc.tile_pool(name="xin", bufs=1) as xin,
            tc.tile_pool(name="io", bufs=4) as io,
            tc.tile_pool(name="work", bufs=3) as work,
            tc.tile_pool(name="small", bufs=4) as small,
            tc.tile_pool(name="ps_t", bufs=2, space="PSUM") as ps_t,
            tc.tile_pool(name="ps_m", bufs=2, space="PSUM") as ps_m,
            tc.tile_pool(name="ps_o", bufs=3, space="PSUM") as ps_o,
        ):
            # ---- weights first on the scalar ring: gate matmuls
            wp = consts.tile([128, 3 * DX], bf16)
            nc.scalar.dma_start(out=wp, in_=wpack[:, :])
            w1_sb = wp[:, 0:128]
            w2c_sb = wp[:, 128:256]
            ident = wp[:, 256:384]

            eps = consts.tile([128, 1], f32)
            nc.vector.memset(eps, 1e-6)

            # ---- xb row-major on sync (2KB/partition descriptors);
            # xcb on the gpsimd SWDGE ring (1KB descriptors)
            xbs = []
            for i in range(4):
                t = xin.tile([128, 8, 128], bf16, tag=f"xb{i}")
                nc.sync.dma_start(out=t, in_=xb_r[:, ts(i, 8), :])
                xbs.append(t)
            xcbs = []
            for i in range(2):
                t = xin.tile([128, 4, _R, 128], bf16, tag=f"xcb{i}")
                nc.gpsimd.dma_start(
                    out=t, in_=xcb[4 * i : 4 * i + 4].rearrange(
                        "g p c d -> p g c d"),
                )
                xcbs.append(t)

            # ---- ACT table warms (Relu/Sqrt/Copy) before data lands
            warm = consts.tile([128, 1], f32)
            nc.scalar.activation(out=warm, in_=eps, func=AF.Relu)
            nc.scalar.activation(out=warm, in_=eps, func=AF.Sqrt, bias=eps)
            nc.scalar.copy(out=warm, in_=eps)

            # ---- PE HAM warmup: real bf16 matmuls on the weight pack
            pewarm = ps_o.tile([128, _R * 128], f32, tag="po")
            for _ in range(4):
                nc.tensor.matmul(pewarm[:, 0 : 3 * DX], lhsT=ident,
                                 rhs=wp[:, :], start=True, stop=True)
            warmsink = consts.tile([128, 1], f32)
            nc.vector.tensor_copy(out=warmsink, in_=pewarm[:, 0:1])

            for g in range(_SLABS):
                xg = xbs[g // 2][:, 4 * (g % 2) : 4 * (g % 2) + 4, :]
                cg = xcbs[g // 4][:, g % 4, :, :]

                # ---- transpose x chunks to (d, p) layout on the PE ----
                xtp = ps_t.tile([128, _R, 128], bf16, tag="xtp")
                for r in range(_R):
                    nc.tensor.transpose(xtp[:, r, :], xg[:, r, :], ident)
                xT = work.tile([128, _R, 128], bf16, tag="xT")
                nc.vector.tensor_copy(out=xT, in_=xtp)

                # ---- mm1: y1 = x @ W1 in (e, r, p) layout ----
                y1p = ps_m.tile([128, _R * 128], f32, tag="y1p")
                nc.tensor.matmul(y1p, lhsT=w1_sb,
                                 rhs=xT.rearrange("d r p -> d (r p)"),
                                 start=True, stop=True)
                y1s = work.tile([128, _R, 128], bf16, tag="y1s")
                nc.scalar.activation(
                    out=y1s.rearrange("e r p -> e (r p)"), in_=y1p, func=AF.Relu
                )

                # ---- po = xc + relu(y1) @ W2C, accumulated in PSUM ----
                po = ps_o.tile([128, _R * 128], f32, tag="po")
                nc.tensor.matmul(po, lhsT=ident,
                                 rhs=cg.rearrange("p r d -> p (r d)"),
                                 start=True, stop=False)
                for r in range(_R):
                    nc.tensor.matmul(po[:, ts(r, 128)], lhsT=y1s[:, r, :],
                                     rhs=w2c_sb, start=False, stop=True)

                # ---- drain po once to SBUF bf16; LN math runs from SBUF
                # at 16-bit DVE/Pool rates (mean(po)=0 by construction).
                pos = work.tile([128, _R, 128], bf16, tag="pos")
                nc.scalar.copy(out=pos.rearrange("p r d -> p (r d)"), in_=po)
                sqb = work.tile([128, _R, 128], bf16, tag="sqb")
                nc.gpsimd.tensor_tensor(out=sqb, in0=pos, in1=pos, op=OP.mult)
                ssq = small.tile([128, _R], f32, tag="ssq")
                nc.vector.tensor_reduce(out=ssq, in_=sqb,
                                        axis=mybir.AxisListType.X, op=OP.add)
                std = small.tile([128, _R], f32, tag="std")
                nc.scalar.activation(out=std, in_=ssq, func=AF.Sqrt,
                                     scale=1.0 / 128.0, bias=eps)
                rstd = small.tile([128, _R], f32, tag="rstd")
                nc.vector.reciprocal(out=rstd, in_=std)
                rstdb = small.tile([128, _R], bf16, tag="rstdb")
                nc.vector.tensor_copy(out=rstdb, in_=rstd)

                # ---- normalize: og = pos * rstd (bf16, all-SBUF) ----
                og = io.tile([128, _R, 128], bf16, tag="og")
                rb = rstdb.to_broadcast([128, _R, 128])
                nc.vector.tensor_tensor(out=og, in0=pos, in1=rb, op=OP.mult)

                # bf16 -> f32 cast during the SWDGE store
                nc.gpsimd.dma_start(out=y_r[:, ts(g, _R), :], in_=og)
    nc.finalize()
    return nc


def _ensure_ntff_hook():
    """Register the axon NTFF profiling hook if the image lacks antenv.axon_hooks."""
    try:
        from antenv.axon_hooks import get_axon_ntff_profile_hook  # noqa: F401
        return
    except ImportError:
        pass
    import sys
    import types

    import antenv
    from trn_agent_boot.trn_boot import _ntff_profile_via_ctypes

    hook = _ntff_profile_via_ctypes("/opt/axon/libaxon_pjrt.so")
    mod = types.ModuleType("antenv.axon_hooks")
    mod._hook = hook
    mod.set_axon_ntff_profile_hook = lambda h: setattr(mod, "_hook", h)
    mod.get_axon_ntff_profile_hook = lambda: mod._hook
    sys.modules["antenv.axon_hooks"] = mod
    antenv.axon_hooks = mod


def _run_device(x, w1, w2, trace=False):
    import ml_dtypes
    import concourse.bass_utils as bass_utils
    from concourse.bass_utils import run_bass_kernel_spmd

    if trace:
        try:
            _ensure_ntff_hook()
            bass_utils.upload_artifacts = lambda tmpdir: str(tmpdir)
        except Exception as e:  # profiling is best-effort
            print(f"ntff hook unavailable ({e}); running without trace")
            trace = False

    if "prog" not in _prog_cache:
        _prog_cache["prog"] = _build_program()
    nc = _prog_cache["prog"]

    bf = ml_dtypes.bfloat16
    x = np.ascontiguousarray(x, dtype=np.float32)
    mu = x.mean(axis=-1, keepdims=True)
    xb16 = x.astype(bf)
    # [B, slab, partition, chunk, d] with row = 32p + 4g + r: reshape
    # (p, k) major then split k -> (g, r) and move g out front.
    xc = (x - mu).astype(bf).reshape(B, 128, _SLABS, _R, DX)
    xcb16 = np.ascontiguousarray(xc.transpose(0, 2, 1, 3, 4))

    w1c = np.ascontiguousarray(w1, dtype=np.float32)
    w2c = np.ascontiguousarray(w2, dtype=np.float32)
    cmat = np.eye(DX, dtype=np.float32) - np.float32(1.0 / DX)
    w2cc = (w2c @ cmat).astype(bf)
    wpack = np.concatenate(
        [w1c.astype(bf), w2cc, np.eye(DX, dtype=np.float32).astype(bf)], axis=1
    )
    wpack = np.ascontiguousarray(wpack)

    in_maps = [
        {
            "xb": np.ascontiguousarray(xb16[b]),
            "xcb": xcb16[b],
            "wpack": wpack,
        }
        for b in range(B)
    ]
    res = run_bass_kernel_spmd(
        nc, in_maps, core_ids=list(range(B)), trace=trace,
        trace_cores=list(range(B)) if trace else None,
    )
    kernel.last_result = res
    kernel.last_exec_time_ns = res.exec_time_ns
    return np.stack([r["y"] for r in res.results], axis=0)


def _numpy_fallback(inputs):
    """Faithful (but slow) mirror of the reference for unexpected inputs."""
    f32 = np.float32
    x = np.asarray(inputs["x"], f32)
    c = np.asarray(inputs["c"], f32)
    W1 = np.asarray(inputs["W1"], f32); W2 = np.asarray(inputs["W2"], f32)
    wt_w = np.asarray(inputs["wt_w"], f32); bsa = np.asarray(inputs["bsa"], f32)
    Wsa1 = np.asarray(inputs["Wsa1"], f32); Wsa2 = np.asarray(inputs["Wsa2"], f32)
    wsat_w = np.asarray(inputs["wsat_w"], f32)
    wsat_b = np.asarray(inputs["wsat_b"], f32); bsa1 = np.asarray(inputs["bsa1"], f32)
    pfn_w1 = np.asarray(inputs["pfn_w1"], f32); pfn_b1 = np.asarray(inputs["pfn_b1"], f32)
    pfn_w2 = np.asarray(inputs["pfn_w2"], f32); pfn_b2 = np.asarray(inputs["pfn_b2"], f32)
    ln_g = np.asarray(inputs["ln_g"], f32); ln_b = np.asarray(inputs["ln_b"], f32)
    Bs, Ls, _ = x.shape
    wx = x @ W1
    wq = c @ W2
    logits = (wx + wq[:, None, :] + bsa) @ wt_w
    m = logits.max(-1, keepdims=True)
    e = np.exp(logits - m)
    p = (e / e.sum(-1, keepdims=True))[..., None]
    h = x * p
    si = (h @ Wsa1) @ wsat_w
    sj = (h @ Wsa2) @ wsat_w
    const = bsa1 @ wsat_w + wsat_b
    colsum = np.zeros((Bs, Ls), f32)
    blk = 512
    for b in range(Bs):
        for i0 in range(0, Ls, blk):
            s = 1.0 / (1.0 + np.exp(-(si[b, i0 : i0 + blk, None] + sj[b, None, :] + const)))
            for r in range(s.shape[0]):
                s[r, i0 + r] = -np.inf
            sm = s.max(-1, keepdims=True)
            ee = np.exp(s - sm)
            colsum[b] += (ee / ee.sum(-1, keepdims=True)).sum(0)
    ui = x * colsum[..., None]
    yv = np.maximum(ui @ pfn_w1 + pfn_b1, 0.0)
    yv = yv @ pfn_w2 + pfn_b2 + ui
    mu = yv.mean(-1, keepdims=True)
    var = ((yv - mu) ** 2).mean(-1, keepdims=True)
    return ((yv - mu) / np.sqrt(var + 1e-6) * ln_g + ln_b).astype(f32)


def kernel(**inputs):
    x = np.asarray(inputs["x"], dtype=np.float32)
    pfn_w1 = np.asarray(inputs["pfn_w1"], dtype=np.float32)
    pfn_w2 = np.asarray(inputs["pfn_w2"], dtype=np.float32)

    fast_ok = (
        x.shape == (B, L, DX)
        and not np.any(np.asarray(inputs["pfn_b1"]))
        and not np.any(np.asarray(inputs["pfn_b2"]))
        and np.all(np.asarray(inputs["ln_g"]) == 1.0)
        and not np.any(np.asarray(inputs["ln_b"]))
    )
    if not fast_ok:
        return _numpy_fallback(inputs)

    trace = bool(int(os.environ.get("CSA_TRACE", "0")))
    return _run_device(x, pfn_w1, pfn_w2, trace=trace)


kernel.last_exec_time_ns = None
kernel.last_result = None

